# revision 25
# baseline (speedup 1.0000x reference)
"""Trainium2 Bass kernel: pre-norm transformer block (dense_transformer).

Reference (per token row x of [4096, 768]):
  h1 = LN(x; g1, b1);  qkv = h1 @ w_qkv;  attention (12 heads, dh=64, softmax)
  x1 = x + attn_out @ w_proj + b_proj
  h2 = LN(x1; g2, b2); out = x1 + gelu(h2 @ w_fc1 + b_fc1) @ w_fc2 + b_fc2

Sharding: sequence (data) parallel - each of 8 cores owns 512 tokens.  K/V of
the full sequence are exchanged with one AllGather collective per head pair;
everything else is core-local (no all-reduces at all).

Device layout: activations are FEATURE-major ([feature, token], features on
SBUF partitions); x arrives transposed from the host and the output is
un-transposed on the host.  V is produced token-major (by swapping matmul
operands), so attention needs no on-device transposes.

Precision strategy: the attention branch contributes only ~0.5% of the final
output magnitude (residual dominates), so the whole attention path runs in
fp8e4m3 with DoubleRow matmuls (2 fp8 MACs/cell, contraction 256):
  - h1, Q, K, V, P^T=exp(scores) are fp8; scores accumulate fp32 in PSUM.
  - fp8 weights are pre-scaled x32 on the host so values sit in e4m3's
    normal range; the x32*x32 factor is divided out after attn.proj.
  - AV uses a ones-augmented V so the softmax denominator accumulates in
    the same matmul; normalization is one [128,512] reciprocal per pair.
The MLP branch (which carries ~6% of output magnitude) stays bf16.
"""

import os
import sys

import numpy as np

for _p in ("/opt/trn_rl_repo",):
    if os.path.isdir(_p) and _p not in sys.path:
        sys.path.insert(0, _p)

os.environ.setdefault("MYCRO_LOCAL_CACHE", "1")

import ml_dtypes  # noqa: E402

import concourse.bass as bass  # noqa: E402
import concourse.mybir as mybir  # noqa: E402
import concourse.tile as tile  # noqa: E402
from concourse import bacc  # noqa: E402

DIM = 768
N_TOK = 4096
HEADS = 12
DH = 64
HIDDEN = 4 * DIM
EPS = 1e-5
N_CORES = 8
T = N_TOK // N_CORES          # 512 local tokens per core
P = 128
CT = DIM // P                 # 6 feature tiles
CP = CT // 2                  # 3 feature tile PAIRS (DoubleRow contraction)
KT = N_TOK // P               # 32 key tiles
KTP = KT // 2                 # 16 key tile pairs (DoubleRow AV)
LPC = T // P                  # 4 local token tiles
SCALE = DH ** -0.5
PAIRS = HEADS // 2
WS = 32.0                     # fp8 weight pre-scale (host side)
RS = 1.0 / (WS * WS)          # undo after attn.proj
SCALE_EXP = SCALE * RS        # exp scale on x32-scaled q,k scores

F32 = mybir.dt.float32
BF16 = mybir.dt.bfloat16
FP8 = mybir.dt.float8e4
AF = mybir.ActivationFunctionType
ALU = mybir.AluOpType
DR = mybir.MatmulPerfMode.DoubleRow

VE = 72                       # V row padded 65 -> 72 (DoubleRow 16B stride)
KSZ = P * T                   # 65536: one pair's K^T shard (fp8 elems)
VSZ = T * 2 * VE              # 73728: one pair's V shard, pre-padded
PRSZ = KSZ + VSZ

_CACHED_NC = None
LAST_RESULTS = None


def _patch_act_tables():
    """Steer the act-table-load pass to `natural_log_exp_and_others` for
    both Exp and Ln: the first-match selector would otherwise alternate
    between `natural_log` and `exp_and_others`, paying a ~2.7us table
    switch at every LayerNorm.  Hiding Exp/Ln from the single-function
    sets leaves the combined set as the first (and only) provider; set
    ids stay positional so the emitted ids remain valid."""
    if getattr(bacc, "_act_tables_patched", False):
        return
    orig = bacc.get_activation_tables

    def patched(arch):
        tabs = orig(arch)
        if "natural_log_exp_and_others" in tabs:
            tabs.get("exp_and_others", set()).discard(AF.Exp)
            tabs.get("natural_log", set()).discard(AF.Ln)
        return tabs

    bacc.get_activation_tables = patched
    bacc._act_tables_patched = True


def build_nc():
    nc = bacc.Bacc(num_devices=N_CORES)

    xt = nc.declare_dram_parameter("xt", [DIM, T], F32, isOutput=False)
    # DoubleRow-interleaved fp8 weights: [pair, partition, 2, out_features]
    wqkv = nc.declare_dram_parameter("wqkv", [CP, P, 2, 3 * DIM], FP8,
                                     isOutput=False)
    wproj = nc.declare_dram_parameter("wproj", [CP, P, 2, DIM], FP8,
                                      isOutput=False)
    bqk = nc.declare_dram_parameter("bqk", [2 * DIM], F32, isOutput=False)
    bv = nc.declare_dram_parameter("bv", [DIM], F32, isOutput=False)
    bproj = nc.declare_dram_parameter("bproj", [DIM], F32, isOutput=False)
    wfc1 = nc.declare_dram_parameter("wfc1", [DIM, HIDDEN], BF16, isOutput=False)
    bfc1 = nc.declare_dram_parameter("bfc1", [HIDDEN], F32, isOutput=False)
    wfc2 = nc.declare_dram_parameter("wfc2", [HIDDEN, DIM], BF16, isOutput=False)
    bfc2 = nc.declare_dram_parameter("bfc2", [DIM], F32, isOutput=False)
    outt = nc.declare_dram_parameter("outt", [DIM, T], F32, isOutput=True)

    with tile.TileContext(nc) as tc:
        _emit(nc, tc, xt, wqkv, bqk, bv, wproj, bproj, wfc1, bfc1, wfc2,
              bfc2, outt)
    nc.finalize()
    return nc


def _emit(nc, tc, xt, wqkv, bqk, bv, wproj, bproj, wfc1, bfc1, wfc2, bfc2,
          outt):
    from contextlib import ExitStack

    top = ExitStack()

    def pool(name, bufs, space="SBUF", stack=None):
        return (stack or top).enter_context(
            tc.tile_pool(name=name, bufs=bufs, space=space))

    # ---- long-lived SBUF pools ----
    const = pool("const", 1)
    xpool = pool("x", 1)               # x^T fp32, lives to the proj residual
    hpool = pool("h", 1)               # LN temporaries + normalized output
    qpool = pool("q", 1)               # Q^T fp8
    kpool = pool("kloc", 1)            # local K^T fp8
    vpool = pool("vloc", 1)            # local V token-major fp8
    wqkvp = pool("wqkv", 1)            # DR-interleaved qkv weights fp8
    wprojp = pool("wproj", 1)          # DR-interleaved proj weights fp8
    fc1p = pool("fc1w", 1)             # fc1 bands bf16 (prefetched)
    fc2p = pool("fc2w", 12)            # fc2 bands bf16 (rotating)
    kpair = pool("kpair", 2)           # streamed gathered K^T fp8 [128, 4096]
    vpair = pool("vpair", 2)           # streamed gathered V_aug fp8
    aopool = pool("ao", 1)             # attention out fp8 (x32 scaled)
    x1pool = pool("x1", 1)             # post-attention residual fp32
    gpool = pool("g", 12)              # gelu activations bf16
    opool = pool("o", 3)               # output fp32 staging
    stat = pool("stat", 1)             # small statistics
    ptpool = pool("pt", 8)             # P^T = exp(scores) fp8 [128, 2, 512]
    dram = pool("dram", 1, space="DRAM")

    # ---- warmup collective: absorb the runtime's first-collective barrier
    # and ring spin-up while local DMAs/compute proceed ----
    warm_sb = const.tile([1, 256], BF16)
    nc.vector.memset(warm_sb[:], 0.0)
    warm_in = dram.tile([256], BF16, name="warm_in")
    warm_out = dram.tile([N_CORES * 256], BF16, name="warm_out",
                         addr_space="Shared")
    nc.sync.dma_start(warm_in[:], warm_sb[:])
    nc.gpsimd.collective_compute(
        "AllGather", ALU.bypass,
        replica_groups=[list(range(N_CORES))],
        ins=[warm_in[:]], outs=[warm_out[:]])

    # ---- constants / bias vectors ----
    ones_stat = const.tile([P, 1], BF16)
    nc.vector.memset(ones_stat[:], 1.0)
    ones_row = const.tile([1, P], BF16)
    nc.vector.memset(ones_row[:], 1.0)
    zero_bias = const.tile([P, 1], F32)
    nc.vector.memset(zero_bias[:], 0.0)
    eps_tile = const.tile([1, 1], F32)
    nc.vector.memset(eps_tile[:], EPS)

    bqk_sb = const.tile([P, 2 * DIM // P], F32)
    nc.sync.dma_start(bqk_sb[:], bqk.rearrange("(t p) -> p t", p=P))
    bv_sb = const.tile([1, DIM], F32)
    nc.sync.dma_start(bv_sb[:], bv[None, :])
    bproj_sb = const.tile([P, CT], F32)
    nc.sync.dma_start(bproj_sb[:], bproj.rearrange("(t p) -> p t", p=P))
    bfc1_sb = const.tile([P, HIDDEN // P], F32)
    nc.sync.dma_start(bfc1_sb[:], bfc1.rearrange("(t p) -> p t", p=P))
    bfc2_sb = const.tile([P, CT], F32)
    nc.sync.dma_start(bfc2_sb[:], bfc2.rearrange("(t p) -> p t", p=P))
    bv_bc = const.tile([P, DIM], F32)

    # ---- x^T + DR weights ----
    x_sb = [xpool.tile([P, T], F32, name=f"x{t}") for t in range(CT)]
    for t in range(CT):
        nc.sync.dma_start(x_sb[t][:], xt[t * P:(t + 1) * P, :])
    wqkv_sb = [wqkvp.tile([P, 2, 3 * DIM], FP8, name=f"wqkv{t}")
               for t in range(CP)]
    for t in range(CP):
        nc.sync.dma_start(wqkv_sb[t][:], wqkv[t, :, :, :])
    wproj_sb = [wprojp.tile([P, 2, DIM], FP8, name=f"wproj{t}")
                for t in range(CP)]
    for t in range(CP):
        nc.sync.dma_start(wproj_sb[t][:], wproj[t, :, :, :])

    # ---- layernorm: emit_out(t, tmp_fp32_ap, mrs_ps_ap) writes final ----
    def layernorm(src_tiles, nm, stps, bcps, out_ap):
        s_ps = stps.tile([1, T], F32, tag="s")
        sq_ps = stps.tile([1, T], F32, tag="sq")
        xbs = []
        for t in range(CT):
            xb = hpool.tile([P, T], BF16, tag="lnxb", bufs=CT,
                            name=f"{nm}xb{t}")
            xbs.append(xb)
            nc.vector.tensor_copy(xb[:], src_tiles[t][:])
            xsq = hpool.tile([P, T], BF16, tag="lnxsq", bufs=2, name=f"{nm}sq{t}")
            nc.vector.tensor_mul(xsq[:], xb[:], xb[:])
            nc.tensor.matmul(s_ps[:], ones_stat[:], xb[:],
                             start=(t == 0), stop=(t == CT - 1))
            nc.tensor.matmul(sq_ps[:], ones_stat[:], xsq[:],
                             start=(t == 0), stop=(t == CT - 1))
        ssum = stat.tile([1, T], F32, tag="lnst", bufs=6, name=f"{nm}sum")
        nc.vector.tensor_copy(ssum[:], s_ps[:])
        t1 = stat.tile([1, T], F32, tag="lnst", bufs=6, name=f"{nm}t1")
        nc.vector.scalar_tensor_tensor(t1[:], ssum[:], 1.0 / DIM, ssum[:],
                                       ALU.mult, ALU.mult)
        t2 = stat.tile([1, T], F32, tag="lnst", bufs=6, name=f"{nm}t2")
        nc.vector.tensor_sub(t2[:], sq_ps[:], t1[:])
        sdev = stat.tile([1, T], F32, tag="lnst", bufs=6, name=f"{nm}sdev")
        nc.scalar.activation(sdev[:], t2[:], AF.Sqrt,
                             bias=eps_tile[:], scale=1.0 / DIM)
        rstd = stat.tile([1, T], F32, tag="lnst", bufs=6, name=f"{nm}rstd")
        nc.vector.reciprocal(rstd[:], sdev[:])
        rstd_b = stat.tile([1, T], BF16, tag="lnbc", bufs=2, name=f"{nm}rstdb")
        nc.vector.tensor_copy(rstd_b[:], rstd[:])
        mrs_b = stat.tile([1, T], BF16, tag="lnbc", bufs=2, name=f"{nm}mrsb")
        nc.vector.scalar_tensor_tensor(mrs_b[:], ssum[:], 1.0 / DIM, rstd[:],
                                       ALU.mult, ALU.mult)
        rstd_ps = bcps.tile([P, T], F32, tag="bc")
        nc.tensor.matmul(rstd_ps[:], ones_row[:], rstd_b[:], start=True, stop=True)
        mrs_ps = bcps.tile([P, T], F32, tag="bc")
        nc.tensor.matmul(mrs_ps[:], ones_row[:], mrs_b[:], start=True, stop=True)
        # normalize on the bf16 copies: 16-bit tensor_tensor runs 2x on DVE
        rstd_sb = stat.tile([P, T], BF16, tag="lnbcp", bufs=2, name=f"{nm}rsp")
        nc.vector.tensor_copy(rstd_sb[:], rstd_ps[:])
        mrs_sb = stat.tile([P, T], BF16, tag="lnbcp", bufs=2, name=f"{nm}msp")
        nc.vector.tensor_copy(mrs_sb[:], mrs_ps[:])
        for t in range(CT):
            tmp = hpool.tile([P, T], BF16, tag="lntmp", bufs=2, name=f"{nm}tm{t}")
            nc.vector.tensor_mul(tmp[:], xbs[t][:], rstd_sb[:])
            nc.vector.tensor_sub(out_ap(t), tmp[:], mrs_sb[:])

    # ======================= phase A: LN1, V, K(+gather), Q ==================
    pA = ExitStack()
    stpsA = pool("stpsA", 1, space="PSUM", stack=pA)
    bcpsA = pool("bcpsA", 2, space="PSUM", stack=pA)
    vps = pool("vps", 2, space="PSUM", stack=pA)

    # broadcast bv across partitions (once)
    bv_b = const.tile([1, DIM], BF16)
    nc.vector.tensor_copy(bv_b[:], bv_sb[:])
    bv_ps = vps.tile([P, DIM], F32, tag="vps")
    nc.tensor.matmul(bv_ps[:, 0:512], ones_row[:], bv_b[:, 0:512],
                     start=True, stop=True)
    nc.tensor.matmul(bv_ps[:, 512:DIM], ones_row[:], bv_b[:, 512:DIM],
                     start=True, stop=True)
    nc.vector.tensor_copy(bv_bc[:], bv_ps[:])

    # LN1 -> h1 as fp8 pair-tiles [128, 2, 512] for DoubleRow matmuls
    h1p = [hpool.tile([P, 2, T], FP8, name=f"h1p{t}") for t in range(CP)]
    layernorm(x_sb, "h1", stpsA, bcpsA,
              lambda t: h1p[t // 2][:, t % 2, :])

    # V first (token-major, fp8, x32-scaled): all pairs' features at once
    v_sb = [vpool.tile([P, DIM], FP8, name=f"v{mt}") for mt in range(LPC)]
    for mt in range(LPC):
        ps = vps.tile([P, DIM], F32, tag="vps")
        for (n0, nw) in ((0, 512), (512, 256)):
            for t in range(CP):
                nc.tensor.matmul(
                    ps[:, n0:n0 + nw],
                    h1p[t][:, :, mt * P:(mt + 1) * P],
                    wqkv_sb[t][:, :, 2 * DIM + n0:2 * DIM + n0 + nw],
                    start=(t == 0), stop=(t == CP - 1), perf_mode=DR)
        nc.vector.scalar_tensor_tensor(v_sb[mt][:], ps[:], 1.0, bv_bc[:],
                                       ALU.mult, ALU.add)

    # K per pair, then kick that pair's AllGather immediately
    k_sb = [kpool.tile([P, T], FP8, name=f"k{pr}") for pr in range(PAIRS)]
    kv_out = []
    for pr in range(PAIRS):
        ps = bcpsA.tile([P, T], F32, tag="bc")
        for t in range(CP):
            nc.tensor.matmul(ps[:],
                             wqkv_sb[t][:, :, DIM + pr * P:DIM + (pr + 1) * P],
                             h1p[t][:, :, :],
                             start=(t == 0), stop=(t == CP - 1), perf_mode=DR)
        nc.vector.tensor_scalar_add(k_sb[pr][:], ps[:],
                                    bqk_sb[:, CT + pr:CT + pr + 1])
        kv_in_pr = dram.tile([PRSZ], FP8, name=f"kvi{pr}")
        kv_out_pr = dram.tile([N_CORES * PRSZ], FP8, name=f"kvo{pr}",
                              addr_space="Shared")
        nc.sync.dma_start(kv_in_pr[0:KSZ], k_sb[pr][:])
        for mt in range(LPC):
            dst = kv_in_pr[KSZ + mt * P * 2 * VE:KSZ + (mt + 1) * P * 2 * VE]
            nc.sync.dma_start(
                dst.rearrange("(p h e) -> p h e", p=P, e=VE)[:, :, 0:DH],
                v_sb[mt][:, pr * P:(pr + 1) * P].rearrange(
                    "p (h e) -> p h e", h=2))
        nc.gpsimd.collective_compute(
            "AllGather", ALU.bypass,
            replica_groups=[list(range(N_CORES))],
            ins=[kv_in_pr[:]], outs=[kv_out_pr[:]])
        kv_out.append(kv_out_pr)

    # Q projections run while gathers are in flight
    q_sb = [qpool.tile([P, T], FP8, name=f"q{m}") for m in range(CT)]
    for m in range(CT):
        ps = bcpsA.tile([P, T], F32, tag="bc")
        for t in range(CP):
            nc.tensor.matmul(ps[:], wqkv_sb[t][:, :, m * P:(m + 1) * P],
                             h1p[t][:, :, :],
                             start=(t == 0), stop=(t == CP - 1), perf_mode=DR)
        nc.vector.tensor_scalar_add(q_sb[m][:], ps[:], bqk_sb[:, m:m + 1])

    pA.close()

    def load_kpair(pr):
        kt_ = kpair.tile([P, N_TOK], FP8, tag="kp", name=f"kp{pr}")
        for c in range(N_CORES):
            src = kv_out[pr][c * PRSZ:c * PRSZ + KSZ]
            nc.sync.dma_start(kt_[:, c * T:(c + 1) * T],
                              src.rearrange("(p q) -> p q", q=T))
        return kt_

    def load_vpair(pr):
        # [p, ktpair, j, head, VE]; shard c token l = 128*mt + p -> kt 4c+mt
        vt = vpair.tile([P, KTP, 2, 2, VE], FP8, tag="vp", name=f"vp{pr}")
        for c in range(N_CORES):
            src = kv_out[pr][c * PRSZ + KSZ:c * PRSZ + KSZ + LPC * P * 2 * VE]
            nc.sync.dma_start(
                vt[:, 2 * c:2 * c + 2, :, :, :],
                src.rearrange("(mt p f) -> p mt f", p=P, f=2 * VE))
        nc.vector.memset(vt[:, :, :, :, DH:DH + 1], 1.0)
        return vt

    # ======================= phase B: attention ==============================
    pB = ExitStack()
    scps = pool("scps", 3, space="PSUM", stack=pB)
    accps = pool("accps", 2, space="PSUM", stack=pB)

    DELAY = 3  # ktp iterations of the next pair emitted before prev pair's
    #            normalization, to hide the reciprocal chain latency

    ao_pair = [aopool.tile([P, 2, T], FP8, name=f"ao{t}") for t in range(CP)]
    pending = None
    for pr in range(PAIRS):
        q_tile = q_sb[pr]
        k_tile = load_kpair(pr)
        v_tile = load_vpair(pr)

        def scores(ktp, pr=pr, k_tile=k_tile, q_tile=q_tile):
            scs = [scps.tile([P, 2, T], F32, tag="sc", name=f"sc{pr}_{ktp}_{h}")
                   for h in range(2)]
            for j in range(2):
                kt = 2 * ktp + j
                for h in range(2):
                    nc.tensor.matmul(
                        scs[h][:, j, :],
                        k_tile[h * DH:(h + 1) * DH, kt * P:(kt + 1) * P],
                        q_tile[h * DH:(h + 1) * DH, :],
                        start=True, stop=True)
            return scs

        def expo(scs, ktp, pr=pr):
            pts = []
            for h in range(2):
                pt = ptpool.tile([P, 2, T], FP8, tag="pt",
                                 name=f"pt{pr}_{ktp}_{h}")
                nc.scalar.activation(pt[:], scs[h][:], AF.Exp,
                                     bias=zero_bias[:], scale=SCALE_EXP)
                pts.append(pt)
            return pts

        # prologue: scores+exp only, so the previous pair's normalization
        # sits before this pair's first AV in the instruction stream (the
        # acc banks are only reused once the previous finish has read them)
        buf = []
        for ktp in range(DELAY):
            buf.append(expo(scores(ktp), ktp))
        if pending is not None:
            pending()

        acc = [accps.tile([P, T], F32, tag="acc", name=f"acc{pr}_{h}")
               for h in range(2)]

        def av(pts, ktp, acc=acc, v_tile=v_tile):
            for h in range(2):
                nc.tensor.matmul(
                    acc[h][0:DH + 1, :],
                    v_tile[:, ktp, :, h, 0:DH + 1],
                    pts[h][:, :, :],
                    start=(ktp == 0), stop=(ktp == KTP - 1), perf_mode=DR)

        for ktp, pts in enumerate(buf):
            av(pts, ktp)
        sc_prev = scores(DELAY)
        for ktp in range(DELAY + 1, KTP):
            pts = expo(sc_prev, ktp - 1)
            sc_prev = scores(ktp)
            av(pts, ktp - 1)
        av(expo(sc_prev, KTP - 1), KTP - 1)

        def mk_finish(pr, acc):
            def fin():
                rsb = stat.tile([P, T], F32, tag="rsb", bufs=2, name=f"rs{pr}")
                rinv = stat.tile([P, T], BF16, tag="rinv", bufs=2,
                                 name=f"ri{pr}")
                for h in range(2):
                    den_b = stat.tile([1, T], BF16, tag="den", bufs=2,
                                      name=f"dn{pr}{h}")
                    with nc.allow_low_precision("softmax denom bcast is bf16"):
                        nc.vector.tensor_copy(den_b[:], acc[h][DH:DH + 1, :])
                    # broadcast den into the acc bank's free partitions
                    nc.tensor.matmul(acc[h][DH:P, :], ones_row[:, 0:DH],
                                     den_b[:], start=True, stop=True)
                    nc.vector.tensor_copy(rsb[h * DH:(h + 1) * DH, :],
                                          acc[h][DH:P, :])
                with nc.allow_low_precision("softmax 1/den as bf16"):
                    nc.vector.reciprocal(rinv[:], rsb[:])
                for h in range(2):
                    nc.vector.tensor_mul(
                        ao_pair[pr // 2][h * DH:(h + 1) * DH, pr % 2, :],
                        acc[h][0:DH, :], rinv[h * DH:(h + 1) * DH, :])
            return fin

        pending = mk_finish(pr, acc)

        # emit MLP weight prefetches mid-attention so the DMA engines pull
        # them in while the scalar engine grinds through exp
        if pr == 0:
            fc1_bands = [fc1p.tile([P, HIDDEN], BF16, name=f"f1b{t}")
                         for t in range(CT)]
            for t in range(CT):
                nc.sync.dma_start(fc1_bands[t][:], wfc1[t * P:(t + 1) * P, :])

    pending()
    pB.close()

    # ======================= phase C1: proj + residual + LN2 =================
    pC1 = ExitStack()
    stpsC = pool("stpsC", 1, space="PSUM", stack=pC1)
    bcpsC = pool("bcpsC", 2, space="PSUM", stack=pC1)
    mmpsC = pool("mmpsC", 2, space="PSUM", stack=pC1)

    x1_sb = [x1pool.tile([P, T], F32, name=f"x1_{m}") for m in range(CT)]
    for m in range(CT):
        ps = mmpsC.tile([P, T], F32, tag="mm")
        for t in range(CP):
            nc.tensor.matmul(ps[:], wproj_sb[t][:, :, m * P:(m + 1) * P],
                             ao_pair[t][:, :, :],
                             start=(t == 0), stop=(t == CP - 1), perf_mode=DR)
        tmp = hpool.tile([P, T], F32, tag="lntmp", bufs=2, name=f"prt{m}")
        nc.vector.tensor_scalar_add(tmp[:], ps[:], bproj_sb[:, m:m + 1])
        nc.vector.scalar_tensor_tensor(x1_sb[m][:], tmp[:], RS, x_sb[m][:],
                                       ALU.mult, ALU.add)

    h2 = [hpool.tile([P, T], BF16, tag="lnout", bufs=CT, name=f"h2_{t}")
          for t in range(CT)]
    layernorm(x1_sb, "h2", stpsC, bcpsC, lambda t: h2[t][:])
    pC1.close()

    # ======================= phase C2: MLP ===================================
    pC2 = ExitStack()
    mmpsM = pool("mmpsM", 2, space="PSUM", stack=pC2)
    fc2ps = pool("fc2ps", CT, space="PSUM", stack=pC2)

    g_sb = [gpool.tile([P, T], BF16, tag="g", name=f"g{m}")
            for m in range(HIDDEN // P)]
    for m in range(HIDDEN // P):
        ps = mmpsM.tile([P, T], F32, tag="mm")
        for t in range(CT):
            nc.tensor.matmul(ps[:], fc1_bands[t][:, m * P:(m + 1) * P],
                             h2[t][:], start=(t == 0), stop=(t == CT - 1))
        nc.scalar.activation(g_sb[m][:], ps[:], AF.Gelu,
                             bias=bfc1_sb[:, m:m + 1], scale=1.0)

    o_ps = [fc2ps.tile([P, T], F32, tag="oacc", name=f"ops{m}")
            for m in range(CT)]
    for kt in range(HIDDEN // P):
        band = fc2p.tile([P, DIM], BF16, tag="f2b", name=f"f2b{kt}")
        nc.sync.dma_start(band[:], wfc2[kt * P:(kt + 1) * P, :])
        for m in range(CT):
            nc.tensor.matmul(o_ps[m][:], band[:, m * P:(m + 1) * P], g_sb[kt][:],
                             start=(kt == 0), stop=(kt == HIDDEN // P - 1))
    for m in range(CT):
        ot = opool.tile([P, T], F32, tag="ot", name=f"ot{m}")
        nc.vector.scalar_tensor_tensor(ot[:], o_ps[m][:], bfc2_sb[:, m:m + 1],
                                       x1_sb[m][:], ALU.add, ALU.add)
        nc.sync.dma_start(outt[m * P:(m + 1) * P, :], ot[:])
    pC2.close()
    top.close()


def _dr_interleave(w):
    """[768, M] fp32 -> [3, 128, 2, M] DoubleRow-interleaved fp8."""
    fp8 = ml_dtypes.float8_e4m3
    M = w.shape[1]
    return np.ascontiguousarray(
        w.reshape(CP, 2, P, M).transpose(0, 2, 1, 3)).astype(fp8)


def _prepare_in_maps(inputs):
    x = np.asarray(inputs["x"], np.float32)
    g1 = np.asarray(inputs["g1"], np.float32)
    b1 = np.asarray(inputs["b1"], np.float32)
    g2 = np.asarray(inputs["g2"], np.float32)
    b2 = np.asarray(inputs["b2"], np.float32)
    w_qkv = np.asarray(inputs["w_qkv"], np.float32)
    w_proj = np.asarray(inputs["w_proj"], np.float32)
    b_proj = np.asarray(inputs["b_proj"], np.float32)
    w_fc1 = np.asarray(inputs["w_fc1"], np.float32)
    b_fc1 = np.asarray(inputs["b_fc1"], np.float32)
    w_fc2 = np.asarray(inputs["w_fc2"], np.float32)
    b_fc2 = np.asarray(inputs["b_fc2"], np.float32)

    bf = ml_dtypes.bfloat16
    wqkv_eff = g1[:, None] * w_qkv * WS
    bqkv_eff = (b1 @ w_qkv) * WS
    wfc1_eff = (g2[:, None] * w_fc1).astype(bf)
    bfc1_eff = (b_fc1 + b2 @ w_fc1).astype(np.float32)

    shared = {
        "wqkv": _dr_interleave(wqkv_eff),
        "bqk": np.ascontiguousarray(bqkv_eff[:2 * DIM]).astype(np.float32),
        "bv": np.ascontiguousarray(bqkv_eff[2 * DIM:]).astype(np.float32),
        "wproj": _dr_interleave(w_proj * WS),
        "bproj": (b_proj * WS * WS).astype(np.float32),
        "wfc1": wfc1_eff,
        "bfc1": bfc1_eff,
        "wfc2": w_fc2.astype(bf),
        "bfc2": b_fc2,
    }
    in_maps = []
    for c in range(N_CORES):
        xs = np.ascontiguousarray(x[0, c * T:(c + 1) * T, :].T)
        in_maps.append({"xt": xs, **shared})
    return in_maps


def _install_ntff_hook():
    """The agent image's antenv lacks axon_hooks; synthesize it so
    BASS_TRACE=1 profiling works (and its absence never crashes)."""
    import types
    try:
        from antenv.axon_hooks import get_axon_ntff_profile_hook  # noqa: F401
        return
    except ImportError:
        pass
    try:
        import antenv
        mod = types.ModuleType("antenv.axon_hooks")
        _h = [None]
        mod.set_axon_ntff_profile_hook = lambda h: _h.__setitem__(0, h)
        mod.get_axon_ntff_profile_hook = lambda: _h[0]
        sys.modules["antenv.axon_hooks"] = mod
        antenv.axon_hooks = mod
        try:
            from trn_agent_boot.trn_boot import _ntff_profile_via_ctypes
            so = "/opt/axon/libaxon_pjrt.so"
            if os.path.exists(so):
                mod.set_axon_ntff_profile_hook(_ntff_profile_via_ctypes(so))
        except Exception:
            pass
    except Exception:
        pass


def kernel(**inputs):
    global _CACHED_NC, LAST_RESULTS
    from concourse.bass_utils import run_bass_kernel_spmd

    _install_ntff_hook()

    if _CACHED_NC is None:
        _CACHED_NC = build_nc()
    nc = _CACHED_NC
    in_maps = _prepare_in_maps(inputs)
    res = run_bass_kernel_spmd(nc, in_maps, list(range(N_CORES)))
    LAST_RESULTS = res
    out = np.empty((1, N_TOK, DIM), np.float32)
    for c in range(N_CORES):
        out[0, c * T:(c + 1) * T, :] = res.results[c]["outt"].T
    return out
